# revision 2
# baseline (speedup 1.0000x reference)
"""Fused single-launch Trainium2 Bass kernel for nn_DeepBiRNN.

B=32, T=1024, D=H=512, L=2, OUT=1024.

Strategy: ONE SPMD launch on 8 cores. Core c handles (dir, batch-slice):
cores 0-3 = fwd slices 0-3, cores 4-7 = bwd slices 0-3 (8 batch rows each).
Each core runs, software-pipelined with per-step lag inside one hw loop:
    L1 step r+4   (gates1 = Wi1@x[r+4] + Wh1@h1 + b1)
    L2 step r+3   (gates2 = Wi2@h1 + Wh2@h2 + b2)
    proj step r+2 (partial out = Wo_half^T @ relu(h2))
All matmuls are weight-stationary [128,128] tiles with 8-wide moving
operands (the batch slice). Gate layout: PSUM [128, 128], partition =
gate_dim % 128, free = (G*4 + hb)*8 + b with G in (i, f, o, g) order.
h lives as [128, 4*8] bf16 = directly the moving operand of the next
step and of the consumer layer. The Wh matmuls are emitted in region
order i, f, g, o so only sigmoid(o) and the h-multiply trail the last
matmul. Partial projections (fwd and bwd halves of W_out) are summed on
the host, which also adds biases b_out.
"""

import numpy as np
import ml_dtypes

import concourse.bacc as bacc
import concourse.mybir as mybir
import concourse.tile as tile
from concourse.bass_utils import run_bass_kernel_spmd


def _launch_global(nc, gin, n_cores=8):
    """Run a compiled Bass module via PJRT with pre-concatenated global
    inputs. Unlike run_bass_via_pjrt, the donated output buffers are
    created directly on device (no zero-upload through the axon tunnel).

    gin: dict name -> global np array of shape (n_cores*d0, *rest).
    Returns dict name -> global np array (n_cores*d0, *rest).
    """
    import jax
    import jax.numpy as jnp
    from jax.sharding import Mesh, PartitionSpec, NamedSharding
    from jax.experimental.shard_map import shard_map
    from concourse.bass2jax import (
        _bass_exec_p, install_neuronx_cc_hook, partition_id_tensor)

    try:
        jax.config.update("jax_compilation_cache_dir",
                          "/root/.jax_comp_cache")
        jax.config.update("jax_persistent_cache_min_compile_time_secs", 0.0)
        jax.config.update("jax_persistent_cache_min_entry_size_bytes", -1)
    except Exception:
        pass
    install_neuronx_cc_hook()
    partition_name = (nc.partition_id_tensor.name
                      if nc.partition_id_tensor else None)
    in_names, out_names, out_avals = [], [], []
    for alloc in nc.m.functions[0].allocations:
        if not isinstance(alloc, mybir.MemoryLocationSet):
            continue
        name = alloc.memorylocations[0].name
        if alloc.kind == "ExternalInput":
            if name != partition_name:
                in_names.append(name)
        elif alloc.kind == "ExternalOutput":
            out_names.append(name)
            out_avals.append(jax.core.ShapedArray(
                tuple(alloc.tensor_shape), mybir.dt.np(alloc.dtype)))
    n_params = len(in_names)
    n_outs = len(out_names)
    all_in_names = list(in_names) + list(out_names)
    if partition_name is not None:
        all_in_names.append(partition_name)
    donate = tuple(range(n_params, n_params + n_outs))

    def _body(*args):
        operands = list(args)
        if partition_name is not None:
            operands.append(partition_id_tensor())
        outs = _bass_exec_p.bind(
            *operands,
            out_avals=tuple(out_avals),
            in_names=tuple(all_in_names),
            out_names=tuple(out_names),
            lowering_input_output_aliases=(),
            sim_require_finite=True,
            sim_require_nnan=True,
            nc=nc,
        )
        return tuple(outs)

    devices = jax.devices()[:n_cores]
    mesh = Mesh(np.asarray(devices), ("core",))
    P = PartitionSpec
    sharded = jax.jit(
        shard_map(_body, mesh=mesh,
                  in_specs=(P("core"),) * (n_params + n_outs),
                  out_specs=(P("core"),) * n_outs, check_rep=False),
        donate_argnums=donate, keep_unused=True)
    zsh = NamedSharding(mesh, P("core"))
    zeros = [
        jax.jit(lambda a=a: jnp.zeros((n_cores * a.shape[0], *a.shape[1:]),
                                      a.dtype), out_shardings=zsh)()
        for a in out_avals
    ]
    out_arrs = sharded(*[gin[n] for n in in_names], *zeros)
    return {
        name: np.asarray(out_arrs[i]).reshape(
            n_cores, *out_avals[i].shape)
        for i, name in enumerate(out_names)
    }

BF16 = ml_dtypes.bfloat16
B, T, D, H = 32, 1024, 512, 512
NCORES = 8
W = 8            # batch rows per core
PRO = 4          # prologue L1 steps
K = 10           # rounds per For_i body
NB = (T - PRO) // K  # 102 body iterations
assert PRO + K * NB == T

_cache = {}

AF = mybir.ActivationFunctionType


def build_fused(debug=False):
    nc = bacc.Bacc("TRN2", target_bir_lowering=False, debug=False,
                   num_devices=NCORES)
    dt = mybir.dt
    wi1 = nc.dram_tensor("wi1", [128, 64 * 128], dt.bfloat16, kind="ExternalInput")
    wh1 = nc.dram_tensor("wh1", [128, 64 * 128], dt.bfloat16, kind="ExternalInput")
    wi2 = nc.dram_tensor("wi2", [128, 64 * 128], dt.bfloat16, kind="ExternalInput")
    wh2 = nc.dram_tensor("wh2", [128, 64 * 128], dt.bfloat16, kind="ExternalInput")
    wo = nc.dram_tensor("wo", [128, 32 * 128], dt.bfloat16, kind="ExternalInput")
    ident = nc.dram_tensor("ident", [128, 128], dt.bfloat16, kind="ExternalInput")
    b1 = nc.dram_tensor("b1", [128, 128], dt.bfloat16, kind="ExternalInput")
    b2 = nc.dram_tensor("b2", [128, 128], dt.bfloat16, kind="ExternalInput")
    xp = nc.dram_tensor("xp", [128, PRO * 32], dt.bfloat16, kind="ExternalInput")
    xb = nc.dram_tensor("xb", [NB, 128, K * 32], dt.bfloat16, kind="ExternalInput")
    h10 = nc.dram_tensor("h10", [128, 32], dt.bfloat16, kind="ExternalInput")
    c10 = nc.dram_tensor("c10", [128, 32], dt.float32, kind="ExternalInput")
    h20 = nc.dram_tensor("h20", [128, 32], dt.bfloat16, kind="ExternalInput")
    c20 = nc.dram_tensor("c20", [128, 32], dt.float32, kind="ExternalInput")
    outp = nc.dram_tensor("outp", [128, 2 * 64], dt.bfloat16, kind="ExternalOutput")
    outb = nc.dram_tensor("outb", [NB, 128, K * 64], dt.bfloat16,
                          kind="ExternalOutput")
    oute = nc.dram_tensor("oute", [128, 2 * 64], dt.bfloat16, kind="ExternalOutput")
    if debug:
        h1p = nc.dram_tensor("h1p", [128, PRO * 32], dt.bfloat16,
                             kind="ExternalOutput")
        h1b = nc.dram_tensor("h1b", [NB, 128, K * 32], dt.bfloat16,
                             kind="ExternalOutput")

    with tile.TileContext(nc) as tc:
        with (
            tc.tile_pool(name="const", bufs=1) as constp,
            tc.tile_pool(name="state", bufs=1) as statep,
            tc.tile_pool(name="xs", bufs=2) as xpool,
            tc.tile_pool(name="stg", bufs=2) as spool,
            tc.tile_pool(name="cell", bufs=2) as cellp,
            tc.tile_pool(name="ps1", bufs=2, space="PSUM") as psp1,
            tc.tile_pool(name="ps2", bufs=2, space="PSUM") as psp2,
            tc.tile_pool(name="psP", bufs=2, space="PSUM") as pspP,
        ):
            def cload(name, t, sz, dtp):
                sb = constp.tile([128, sz], dtp, tag=name)
                nc.sync.dma_start(sb[:], t.ap())
                return sb

            wi1_sb = cload("wi1", wi1, 64 * 128, dt.bfloat16)
            wh1_sb = cload("wh1", wh1, 64 * 128, dt.bfloat16)
            wi2_sb = cload("wi2", wi2, 64 * 128, dt.bfloat16)
            wh2_sb = cload("wh2", wh2, 64 * 128, dt.bfloat16)
            wo_sb = cload("wo", wo, 32 * 128, dt.bfloat16)
            id_sb = cload("ident", ident, 128, dt.bfloat16)
            b1_sb = cload("b1", b1, 128, dt.bfloat16)
            b2_sb = cload("b2", b2, 128, dt.bfloat16)
            xp_sb = cload("xp", xp, PRO * 32, dt.bfloat16)

            h1_st = statep.tile([128, 32], dt.bfloat16, tag="h1st")
            nc.sync.dma_start(h1_st[:], h10.ap())
            c1_st = statep.tile([128, 32], dt.float32, tag="c1st")
            nc.sync.dma_start(c1_st[:], c10.ap())
            h2_st = statep.tile([128, 32], dt.bfloat16, tag="h2st")
            nc.sync.dma_start(h2_st[:], h20.ap())
            c2_st = statep.tile([128, 32], dt.float32, tag="c2st")
            nc.sync.dma_start(c2_st[:], c20.ap())
            relu_sb = statep.tile([128, 32], dt.bfloat16, tag="relu")
            nc.vector.memset(relu_sb[:], 0)

            # Wh region emit order: i(0-3), f(4-7), g(12-15), o(8-11) so the
            # o gate finishes last and only sig_o + h-mul trail the matmuls.
            WH_ORDER = [0, 1, 2, 3, 4, 5, 6, 7, 12, 13, 14, 15, 8, 9, 10, 11]

            def lstm_mms(psp, tag, wi_sb, wh_sb, bias_sb, xmov, h_tile):
                ps = psp.tile([128, 512], dt.float32, tag=tag)
                nc.tensor.matmul(ps[:, 0:128], id_sb[:], bias_sb[:],
                                 start=True, stop=False)
                for g in range(16):
                    for k in range(4):
                        idx = g * 4 + k
                        nc.tensor.matmul(
                            ps[:, g * 8:(g + 1) * 8],
                            wi_sb[:, idx * 128:(idx + 1) * 128],
                            xmov[:, k * 8:(k + 1) * 8],
                            start=False, stop=False)
                for g in WH_ORDER:
                    for k in range(4):
                        idx = g * 4 + k
                        nc.tensor.matmul(
                            ps[:, g * 8:(g + 1) * 8],
                            wh_sb[:, idx * 128:(idx + 1) * 128],
                            h_tile[:, k * 8:(k + 1) * 8],
                            start=False, stop=(k == 3))
                return ps

            def lstm_tail(ps, c_st, h_st, tag):
                sig_if = cellp.tile([128, 64], dt.float32, tag=f"sif{tag}")
                nc.scalar.activation(sig_if[:], ps[:, 0:64], AF.Sigmoid)
                tg = cellp.tile([128, 32], dt.float32, tag=f"tg{tag}")
                nc.scalar.activation(tg[:], ps[:, 96:128], AF.Tanh)
                u = cellp.tile([128, 32], dt.float32, tag=f"u{tag}")
                nc.vector.tensor_mul(u[:], sig_if[:, 0:32], tg[:])
                v = cellp.tile([128, 32], dt.float32, tag=f"v{tag}")
                nc.gpsimd.tensor_mul(v[:], sig_if[:, 32:64], c_st[:])
                nc.vector.tensor_add(c_st[:], u[:], v[:])
                th = cellp.tile([128, 32], dt.float32, tag=f"th{tag}")
                nc.scalar.activation(th[:], c_st[:], AF.Tanh)
                sig_o = cellp.tile([128, 32], dt.float32, tag=f"so{tag}")
                nc.scalar.activation(sig_o[:], ps[:, 64:96], AF.Sigmoid)
                nc.gpsimd.tensor_mul(h_st[:], sig_o[:], th[:])

            def proj_mms(stage, scol):
                ps = pspP.tile([128, 512], dt.float32, tag="pj")
                for m in range(8):
                    for k in range(4):
                        idx = m * 4 + k
                        nc.tensor.matmul(
                            ps[:, m * 8:(m + 1) * 8],
                            wo_sb[:, idx * 128:(idx + 1) * 128],
                            relu_sb[:, k * 8:(k + 1) * 8],
                            start=(k == 0), stop=(k == 3))
                nc.scalar.copy(stage[:, scol:scol + 64], ps[:, 0:64])

            def full_round(xmov, do_l2, do_proj, stage, scol, h1cap=None):
                ps1 = lstm_mms(psp1, "g1", wi1_sb, wh1_sb, b1_sb, xmov, h1_st)
                if do_l2:
                    ps2 = lstm_mms(psp2, "g2", wi2_sb, wh2_sb, b2_sb,
                                   h1_st, h2_st)
                if do_proj:
                    proj_mms(stage, scol)
                lstm_tail(ps1, c1_st, h1_st, "1")
                if h1cap is not None:
                    nc.vector.tensor_copy(h1cap, h1_st[:])
                if do_l2:
                    lstm_tail(ps2, c2_st, h2_st, "2")
                    nc.vector.tensor_scalar_max(relu_sb[:], h2_st[:], 0.0)

            # ---- prologue: L1 steps 0..3, L2 steps 0..2, proj steps 0..1
            pstage = spool.tile([128, 2 * 64], dt.bfloat16, tag="pstg")
            if debug:
                h1p_sb = spool.tile([128, PRO * 32], dt.bfloat16, tag="h1ps")
            for t in range(PRO):
                full_round(xp_sb[:, t * 32:(t + 1) * 32],
                           do_l2=(t >= 1), do_proj=(t >= 2),
                           stage=pstage, scol=(t - 2) * 64,
                           h1cap=(h1p_sb[:, t * 32:(t + 1) * 32]
                                  if debug else None))
            nc.sync.dma_start(outp.ap(), pstage[:])
            if debug:
                nc.sync.dma_start(h1p.ap(), h1p_sb[:])

            # ---- body
            xb_ap = xb.ap()
            outb_ap = outb.ap()
            if debug:
                h1b_ap = h1b.ap()
            with tc.For_i(0, NB) as i:
                xt = xpool.tile([128, K * 32], dt.bfloat16, tag="xt")
                nc.sync.dma_start(xt[:], xb_ap[i])
                stage = spool.tile([128, K * 64], dt.bfloat16, tag="stg")
                if debug:
                    h1s = spool.tile([128, K * 32], dt.bfloat16, tag="h1s")
                for s in range(K):
                    full_round(xt[:, s * 32:(s + 1) * 32],
                               do_l2=True, do_proj=True,
                               stage=stage, scol=s * 64,
                               h1cap=(h1s[:, s * 32:(s + 1) * 32]
                                      if debug else None))
                nc.sync.dma_start(outb_ap[i], stage[:])
                if debug:
                    nc.sync.dma_start(h1b_ap[i], h1s[:])

            # ---- epilogue: L2 step 1023, proj steps 1022, 1023
            estage = spool.tile([128, 2 * 64], dt.bfloat16, tag="estg")
            proj_mms(estage, 0)           # proj 1022 (reads old relu_sb)
            ps2 = lstm_mms(psp2, "g2", wi2_sb, wh2_sb, b2_sb, h1_st, h2_st)
            lstm_tail(ps2, c2_st, h2_st, "2")
            nc.vector.tensor_scalar_max(relu_sb[:], h2_st[:], 0.0)
            proj_mms(estage, 64)          # proj 1023
            nc.sync.dma_start(oute.ap(), estage[:])
    nc.compile()
    return nc


# ------------------------------------------------------------- host helpers
def to_bf(x):
    return np.ascontiguousarray(x.astype(np.float32).astype(BF16))


def pack_w(Wm):
    """W [512, 2048] -> [128, 64*128] stationary tiles (g, k); g = G*4+hb,
    G in our order (i, f, o, g); source gate order is (i, f, g, o)."""
    out = np.empty((128, 64 * 128), np.float32)
    src_G = {0: 0, 1: 1, 2: 3, 3: 2}
    for G in range(4):
        gs = src_G[G]
        for hb in range(4):
            g = G * 4 + hb
            for k in range(4):
                idx = g * 4 + k
                out[:, idx * 128:(idx + 1) * 128] = \
                    Wm[k * 128:(k + 1) * 128,
                       gs * 512 + hb * 128:gs * 512 + (hb + 1) * 128]
    return to_bf(out)


def pack_wo(Wo_half):
    """Wo_half [512, 1024] -> [128, 32*128] tiles (m, k)."""
    out = np.empty((128, 32 * 128), np.float32)
    for m in range(8):
        for k in range(4):
            idx = m * 4 + k
            out[:, idx * 128:(idx + 1) * 128] = \
                Wo_half[k * 128:(k + 1) * 128, m * 128:(m + 1) * 128]
    return to_bf(out)


def pack_bias(bvec):
    """b [2048] -> [128, 128] gates-layout tile, broadcast over batch."""
    out = np.empty((128, 128), np.float32)
    src_G = {0: 0, 1: 1, 2: 3, 3: 2}
    for G in range(4):
        gs = src_G[G]
        for hb in range(4):
            g = G * 4 + hb
            col = bvec[gs * 512 + hb * 128:gs * 512 + (hb + 1) * 128]
            out[:, g * 8:(g + 1) * 8] = col[:, None]
    return to_bf(out)


def pack_x(x_slice):
    """x_slice [W, T, 512] (already time-ordered for the direction) ->
    (xp [128, PRO*32], xb [NB, 128, K*32]) with col = s*32 + k*8 + b."""
    a = x_slice.reshape(W, T, 4, 128)
    full = np.ascontiguousarray(np.transpose(a, (3, 1, 2, 0))).reshape(
        128, T * 32)
    full = to_bf(full)
    xp = np.ascontiguousarray(full[:, :PRO * 32])
    xbd = np.ascontiguousarray(
        full[:, PRO * 32:].reshape(128, NB, K * 32).transpose(1, 0, 2))
    return xp, xbd


def pack_state(v_slice, cast_bf):
    """[W, 512] -> [128, 32] col = hb*8 + b."""
    t = v_slice.T.reshape(4, 128, W).transpose(1, 0, 2).reshape(128, 32)
    return to_bf(t) if cast_bf else np.ascontiguousarray(t.astype(np.float32))


def unpack_out(outp_a, outb_a, oute_a):
    """-> [T, 128, 8m, 8b] fp32"""
    arr = np.empty((T, 128, 8, 8), np.float32)
    arr[0:2] = outp_a.astype(np.float32).reshape(128, 2, 8, 8).transpose(
        1, 0, 2, 3)
    arr[2:T - 2] = outb_a.astype(np.float32).reshape(
        NB, 128, K, 8, 8).transpose(0, 2, 1, 3, 4).reshape(T - 4, 128, 8, 8)
    arr[T - 2:] = oute_a.astype(np.float32).reshape(128, 2, 8, 8).transpose(
        1, 0, 2, 3)
    return arr


# ------------------------------------------------------------------- kernel
def kernel(x, h0, c0, Wi_f, Wh_f, b_f, Wi_b, Wh_b, b_b, W_out, b_out):
    x = np.asarray(x, np.float32)
    h0 = np.asarray(h0, np.float32); c0 = np.asarray(c0, np.float32)
    Wi_f = np.asarray(Wi_f, np.float32); Wh_f = np.asarray(Wh_f, np.float32)
    Wi_b = np.asarray(Wi_b, np.float32); Wh_b = np.asarray(Wh_b, np.float32)
    b_f = np.asarray(b_f, np.float32); b_b = np.asarray(b_b, np.float32)
    W_out = np.asarray(W_out, np.float32); b_out = np.asarray(b_out, np.float32)

    if "fused" not in _cache:
        _cache["fused"] = build_fused(debug=False)
    f_nc = _cache["fused"]

    ident = to_bf(np.eye(128, dtype=np.float32))
    packs_f = dict(
        wi1=pack_w(Wi_f[0]), wh1=pack_w(Wh_f[0]),
        wi2=pack_w(Wi_f[1]), wh2=pack_w(Wh_f[1]),
        wo=pack_wo(W_out[:512]), b1=pack_bias(b_f[0]), b2=pack_bias(b_f[1]),
    )
    packs_b = dict(
        wi1=pack_w(Wi_b[0]), wh1=pack_w(Wh_b[0]),
        wi2=pack_w(Wi_b[1]), wh2=pack_w(Wh_b[1]),
        wo=pack_wo(W_out[512:]), b1=pack_bias(b_b[0]), b2=pack_bias(b_b[1]),
    )
    in_maps = []
    for c in range(NCORES):
        fwd = c < 4
        sl = c % 4
        xs = x[sl * 8:(sl + 1) * 8]
        if not fwd:
            xs = xs[:, ::-1]
        xp_a, xb_a = pack_x(xs)
        m = dict(packs_f if fwd else packs_b)
        m.update(
            ident=ident, xp=xp_a, xb=xb_a,
            h10=pack_state(h0[0, sl * 8:(sl + 1) * 8], True),
            c10=pack_state(c0[0, sl * 8:(sl + 1) * 8], False),
            h20=pack_state(h0[1, sl * 8:(sl + 1) * 8], True),
            c20=pack_state(c0[1, sl * 8:(sl + 1) * 8], False),
        )
        in_maps.append(m)
    gin = {name: np.concatenate([in_maps[c][name] for c in range(NCORES)], 0)
           for name in in_maps[0]}

    import time as _time
    t0 = _time.time()
    gout = _launch_global(f_nc, gin, NCORES)
    _cache.setdefault("launch_times", []).append(_time.time() - t0)

    out = np.empty((B, T, 1024), np.float32)
    for c in range(NCORES):
        fwd = c < 4
        sl = c % 4
        arr = unpack_out(gout["outp"][c], gout["outb"][c], gout["oute"][c])
        part = arr.transpose(3, 0, 2, 1).reshape(W, T, 1024)
        if fwd:
            out[sl * 8:(sl + 1) * 8] = part
        else:
            out[sl * 8:(sl + 1) * 8] += part[:, ::-1]
    out += b_out
    return out
